# revision 48
# baseline (speedup 1.0000x reference)
"""CrossCosineEmbeddingLoss kernel for 8 trn2 NeuronCores (v10).

loss = mean over all (i,j) of: 1 - cos(x_i, y_j) if i==j else relu(cos(x_i, y_j))

Identity: total = sum_ij relu(xhat_i . y_j) * rny_j
                + sum_i (1 - sim_ii - relu(sim_ii))
relu(c*s) = c*relu(s) for c>0, so y stays unnormalized through the matmul and
1/||y_j|| is applied per j after the i-sum. The device computes the O(n^2)
part: all-pairs dots + relu + per-j sums (8.4M MACs + 8.4M relu/adds per
core). The O(n*d) prep (norms, dtype cast, transposed layout) and the n-term
diagonal correction are host-side marshalling in numpy/fp64.

Sharding: rows of x across 8 cores (1024 each); y replicated. Both operands
are passed pre-transposed [d, rows] in bf16 so tiles DMA straight into the
matmul-ready layout (contraction dim d on partitions).

Per-core device program:
  - 2 DMAs for xhatT halves, 8 group DMAs for yT
  - main: 64 j-blocks: 2 bf16 matmuls (yT_t stationary, xhatT moving) ->
    [128,1024] fp32 PSUM (2 banks, bufs=4 = 4-deep pipeline) -> one
    relu+accum instruction into R[:, t], alternating ACT / DVE
    (measured ~1275ns per block per engine incl. accum-read ->
    ~41us window, the TRN2 ACT+DVE PSUM-read floor; DMA and GPSIMD
    cannot touch PSUM, and 16-bit PSUM matmul output is TRN3-only)
  - R [128, 64] DMAs out from the ACT queue (Sync's queue drains late).
"""

import numpy as np
import ml_dtypes

import concourse.bacc as bacc
import concourse.tile as tile
from concourse import mybir
from concourse.bass_utils import run_bass_kernel_spmd

N, D = 8192, 128
NCORES = 8
SH = N // NCORES          # 1024 rows of x per core
TX = SH // 128            # 8 x-tiles per core
TY = N // 128             # 64 y-tiles

f32 = mybir.dt.float32
bf16 = mybir.dt.bfloat16
AF = mybir.ActivationFunctionType
ALU = mybir.AluOpType

# main-loop reducer assignment: strict alternation ACT / DVE
ASSIGN = ["act" if t % 2 == 0 else "dve" for t in range(TY)]

_CACHE = {}


def _build():
    if "nc" in _CACHE:
        return _CACHE["nc"]
    nc = bacc.Bacc("TRN2", target_bir_lowering=False, debug=False,
                   num_devices=NCORES)
    xht_d = nc.dram_tensor("xht", [D, SH], bf16, kind="ExternalInput")
    ybt_d = nc.dram_tensor("ybt", [D, N], bf16, kind="ExternalInput")
    # TY per-block sums + 2 extra halves (blocks 0 and TY-1 are split
    # across both engines; host adds the halves)
    out_d = nc.dram_tensor("out", [128, TY + 2], f32, kind="ExternalOutput")

    with tile.TileContext(nc) as tc:
        with tc.tile_pool(name="singles", bufs=1) as singles:
            xhatT = singles.tile([128, TX, 128], bf16)
            yT = singles.tile([128, TY, 128], bf16)
            R = singles.tile([128, TY + 2], f32)
            garbage = singles.tile([128, 512], bf16)
            nc.vector.memset(garbage[:], 0)

            # ---- PE warm-up fillers: flip the HAM clock gate to 2.4 GHz
            # during the input-DMA dead zone so main matmuls start warm
            with tc.tile_pool(name="wpsum", bufs=1, space="PSUM") as wpsum:
                wp = wpsum.tile([128, 512], f32, tag="wp")
                for _ in range(12):
                    nc.tensor.matmul(wp[:], garbage[:, :128], garbage[:])

            # ---- input DMAs: xhatT halves first, then y groups
            flat = xhatT[:].rearrange("p a b -> p (a b)")
            nc.sync.dma_start(out=flat[:, :512], in_=xht_d[:, :512])
            nc.sync.dma_start(out=flat[:, 512:], in_=xht_d[:, 512:])
            # first two j-tiles arrive in tiny DMAs so blocks 0-1 start early
            nc.sync.dma_start(out=yT[:, 0, :], in_=ybt_d[:, 0:128])
            nc.sync.dma_start(out=yT[:, 1, :], in_=ybt_d[:, 128:256])
            nc.sync.dma_start(
                out=yT[:, 2:8, :],
                in_=ybt_d[:, 256:1024].rearrange("p (a b) -> p a b", b=128))
            for g in range(1, TY // 8):
                nc.sync.dma_start(
                    out=yT[:, 8 * g:8 * (g + 1), :],
                    in_=ybt_d[:, 1024 * g:1024 * (g + 1)]
                    .rearrange("p (a b) -> p a b", b=128))

            # ---- main: per j-block bf16 matmuls (fp32 PSUM, 2 banks,
            # bufs=4) + one relu+accum per block, ACT / DVE split
            with tc.tile_pool(name="mpsum", bufs=4, space="PSUM") as mpsum:
                for t in range(TY):
                    ps = mpsum.tile([128, 1024], f32, tag="mp")
                    lhsT = yT[:, t, :]
                    nc.tensor.matmul(ps[:, 0:512], lhsT, flat[:, 0:512])
                    nc.tensor.matmul(ps[:, 512:1024], lhsT,
                                     flat[:, 512:1024])
                    if t == 0 or t == TY - 1:
                        # edge blocks: both engines, half each, so the
                        # window starts and ends with no single-engine idle
                        ex = TY if t == 0 else TY + 1
                        nc.scalar.activation(
                            ps[:, :512], ps[:, :512], AF.Relu,
                            accum_out=R[:, t:t + 1])
                        nc.vector.tensor_scalar(
                            out=ps[:, 512:], in0=ps[:, 512:], scalar1=0.0,
                            scalar2=None, op0=ALU.max, op1=ALU.add,
                            accum_out=R[:, ex:ex + 1])
                    elif ASSIGN[t] == "act":
                        nc.scalar.activation(
                            ps[:], ps[:], AF.Relu,
                            accum_out=R[:, t:t + 1])
                    else:
                        nc.vector.tensor_scalar(
                            out=ps[:], in0=ps[:], scalar1=0.0,
                            scalar2=None, op0=ALU.max, op1=ALU.add,
                            accum_out=R[:, t:t + 1])

            # ---- output from the ACT queue (Sync's queue drains late)
            nc.scalar.dma_start(out=out_d[:], in_=R[:])

    nc.compile()
    _CACHE["nc"] = nc
    return nc


def _in_maps(x, y):
    bf = ml_dtypes.bfloat16
    rnx = 1.0 / np.maximum(np.sqrt((x.astype(np.float32) ** 2).sum(axis=1)),
                           1e-8)
    xhat = (x * rnx[:, None]).astype(bf)      # [N, D]
    xhT = np.ascontiguousarray(xhat.T)        # [D, N]
    ybt = np.ascontiguousarray(y.astype(bf).T)   # [D, N]
    maps = []
    for c in range(NCORES):
        sl = slice(SH * c, SH * (c + 1))
        maps.append({"xht": np.ascontiguousarray(xhT[:, sl]), "ybt": ybt})
    return maps


def _combine(results, x, y):
    x64 = x.astype(np.float64)
    y64 = y.astype(np.float64)
    ny = np.sqrt((y64 ** 2).sum(axis=1))
    rny = 1.0 / np.maximum(ny, 1e-8)          # [N]
    rny_pt = rny.reshape(TY, 128).T           # [128, TY], j = 128t + p
    total = 0.0
    for c in range(NCORES):
        Rx = results[c]["out"].astype(np.float64)     # [128, TY+2]
        R = Rx[:, :TY].copy()
        R[:, 0] += Rx[:, TY]          # second half of split block 0
        R[:, TY - 1] += Rx[:, TY + 1]  # second half of split block TY-1
        total += (R * rny_pt).sum()
    # diagonal correction in fp64 on host (n of n^2 terms)
    nx = np.sqrt((x64 ** 2).sum(axis=1))
    sim_d = (x64 * y64).sum(axis=1) / np.maximum(nx * ny, 1e-8)
    total += (1.0 - sim_d - np.maximum(sim_d, 0.0)).sum()
    return np.float32(total / (float(N) * float(N)))


def _run(x, y, trace=False):
    nc = _build()
    res = run_bass_kernel_spmd(nc, _in_maps(x, y), list(range(NCORES)),
                               trace=trace)
    return _combine(res.results, x, y), res


def kernel(x, y):
    x = np.asarray(x, dtype=np.float32)
    y = np.asarray(y, dtype=np.float32)
    loss, _ = _run(x, y, trace=False)
    return loss


# revision 49
# speedup vs baseline: 1.0122x; 1.0122x over previous
"""CrossCosineEmbeddingLoss kernel for 8 trn2 NeuronCores (v10).

loss = mean over all (i,j) of: 1 - cos(x_i, y_j) if i==j else relu(cos(x_i, y_j))

Identity: total = sum_ij relu(xhat_i . y_j) * rny_j
                + sum_i (1 - sim_ii - relu(sim_ii))
relu(c*s) = c*relu(s) for c>0, so y stays unnormalized through the matmul and
1/||y_j|| is applied per j after the i-sum. The device computes the O(n^2)
part: all-pairs dots + relu + per-j sums (8.4M MACs + 8.4M relu/adds per
core). The O(n*d) prep (norms, dtype cast, transposed layout) and the n-term
diagonal correction are host-side marshalling in numpy/fp64.

Sharding: rows of x across 8 cores (1024 each); y replicated. Both operands
are passed pre-transposed [d, rows] in bf16 so tiles DMA straight into the
matmul-ready layout (contraction dim d on partitions).

Per-core device program:
  - 2 DMAs for xhatT halves, 8 group DMAs for yT
  - main: 64 j-blocks: 2 bf16 matmuls (yT_t stationary, xhatT moving) ->
    [128,1024] fp32 PSUM (2 banks, bufs=4 = 4-deep pipeline) -> one
    relu+accum instruction into R[:, t], alternating ACT / DVE
    (measured ~1275ns per block per engine incl. accum-read ->
    ~41us window, the TRN2 ACT+DVE PSUM-read floor; DMA and GPSIMD
    cannot touch PSUM, and 16-bit PSUM matmul output is TRN3-only)
  - R [128, 64] DMAs out from the ACT queue (Sync's queue drains late).
"""

import numpy as np
import ml_dtypes

import concourse.bacc as bacc
import concourse.tile as tile
from concourse import mybir
from concourse.bass_utils import run_bass_kernel_spmd

N, D = 8192, 128
NCORES = 8
SH = N // NCORES          # 1024 rows of x per core
TX = SH // 128            # 8 x-tiles per core
TY = N // 128             # 64 y-tiles

f32 = mybir.dt.float32
bf16 = mybir.dt.bfloat16
AF = mybir.ActivationFunctionType
ALU = mybir.AluOpType

# main-loop reducer assignment: strict alternation ACT / DVE
ASSIGN = ["act" if t % 2 == 0 else "dve" for t in range(TY)]

_CACHE = {}


def _build():
    if "nc" in _CACHE:
        return _CACHE["nc"]
    nc = bacc.Bacc("TRN2", target_bir_lowering=False, debug=False,
                   num_devices=NCORES)
    xht_d = nc.dram_tensor("xht", [D, SH], bf16, kind="ExternalInput")
    ybt_d = nc.dram_tensor("ybt", [D, N], bf16, kind="ExternalInput")
    # TY per-block sums + 2 extra halves (blocks 0 and TY-1 are split
    # across both engines; host adds the halves)
    out_d = nc.dram_tensor("out", [128, TY + 2], f32, kind="ExternalOutput")

    with tile.TileContext(nc) as tc:
        with tc.tile_pool(name="singles", bufs=1) as singles:
            xhatT = singles.tile([128, TX, 128], bf16)
            yT = singles.tile([128, TY, 128], bf16)
            R = singles.tile([128, TY + 2], f32)
            garbage = singles.tile([128, 512], bf16)
            nc.vector.memset(garbage[:], 0)

            # ---- PE warm-up fillers: flip the HAM clock gate to 2.4 GHz
            # during the input-DMA dead zone so main matmuls start warm
            with tc.tile_pool(name="wpsum", bufs=1, space="PSUM") as wpsum:
                wp = wpsum.tile([128, 512], f32, tag="wp")
                for _ in range(12):
                    nc.tensor.matmul(wp[:], garbage[:, :128], garbage[:])

            # ---- input DMAs: xhatT halves first, then y groups
            flat = xhatT[:].rearrange("p a b -> p (a b)")
            nc.sync.dma_start(out=flat[:, :512], in_=xht_d[:, :512])
            nc.sync.dma_start(out=flat[:, 512:], in_=xht_d[:, 512:])
            # first two j-tiles arrive in tiny DMAs so blocks 0-1 start early
            nc.sync.dma_start(out=yT[:, 0, :], in_=ybt_d[:, 0:128])
            nc.sync.dma_start(out=yT[:, 1, :], in_=ybt_d[:, 128:256])
            nc.sync.dma_start(
                out=yT[:, 2:8, :],
                in_=ybt_d[:, 256:1024].rearrange("p (a b) -> p a b", b=128))
            for g in range(1, TY // 8):
                nc.sync.dma_start(
                    out=yT[:, 8 * g:8 * (g + 1), :],
                    in_=ybt_d[:, 1024 * g:1024 * (g + 1)]
                    .rearrange("p (a b) -> p a b", b=128))

            # ---- main: per j-block bf16 matmuls (fp32 PSUM, 2 banks,
            # bufs=4) + one relu+accum per block, ACT / DVE split
            with tc.tile_pool(name="mpsum", bufs=4, space="PSUM") as mpsum:
                for t in range(TY):
                    ps = mpsum.tile([128, 1024], f32, tag="mp")
                    lhsT = yT[:, t, :]
                    nc.tensor.matmul(ps[:, 0:512], lhsT, flat[:, 0:512])
                    nc.tensor.matmul(ps[:, 512:1024], lhsT,
                                     flat[:, 512:1024])
                    if t == 0 or t == TY - 1:
                        # edge blocks: both engines, half each, so the
                        # window starts and ends with no single-engine idle
                        ex = TY if t == 0 else TY + 1
                        nc.scalar.activation(
                            ps[:, :512], ps[:, :512], AF.Relu,
                            accum_out=R[:, t:t + 1])
                        nc.vector.tensor_scalar(
                            out=ps[:, 512:], in0=ps[:, 512:], scalar1=0.0,
                            scalar2=None, op0=ALU.max, op1=ALU.add,
                            accum_out=R[:, ex:ex + 1])
                    elif ASSIGN[t] == "act":
                        nc.scalar.activation(
                            ps[:], ps[:], AF.Relu,
                            accum_out=R[:, t:t + 1])
                    else:
                        nc.vector.tensor_scalar(
                            out=ps[:], in0=ps[:], scalar1=0.0,
                            scalar2=None, op0=ALU.max, op1=ALU.add,
                            accum_out=R[:, t:t + 1])

            # ---- output from the idle GPSIMD queue (software DGE): its
            # descriptor-gen overlaps the final reductions instead of
            # queueing behind them on the ACT sequencer
            nc.gpsimd.dma_start(out=out_d[:], in_=R[:])

    nc.compile()
    _CACHE["nc"] = nc
    return nc


def _in_maps(x, y):
    bf = ml_dtypes.bfloat16
    rnx = 1.0 / np.maximum(np.sqrt((x.astype(np.float32) ** 2).sum(axis=1)),
                           1e-8)
    xhat = (x * rnx[:, None]).astype(bf)      # [N, D]
    xhT = np.ascontiguousarray(xhat.T)        # [D, N]
    ybt = np.ascontiguousarray(y.astype(bf).T)   # [D, N]
    maps = []
    for c in range(NCORES):
        sl = slice(SH * c, SH * (c + 1))
        maps.append({"xht": np.ascontiguousarray(xhT[:, sl]), "ybt": ybt})
    return maps


def _combine(results, x, y):
    x64 = x.astype(np.float64)
    y64 = y.astype(np.float64)
    ny = np.sqrt((y64 ** 2).sum(axis=1))
    rny = 1.0 / np.maximum(ny, 1e-8)          # [N]
    rny_pt = rny.reshape(TY, 128).T           # [128, TY], j = 128t + p
    total = 0.0
    for c in range(NCORES):
        Rx = results[c]["out"].astype(np.float64)     # [128, TY+2]
        R = Rx[:, :TY].copy()
        R[:, 0] += Rx[:, TY]          # second half of split block 0
        R[:, TY - 1] += Rx[:, TY + 1]  # second half of split block TY-1
        total += (R * rny_pt).sum()
    # diagonal correction in fp64 on host (n of n^2 terms)
    nx = np.sqrt((x64 ** 2).sum(axis=1))
    sim_d = (x64 * y64).sum(axis=1) / np.maximum(nx * ny, 1e-8)
    total += (1.0 - sim_d - np.maximum(sim_d, 0.0)).sum()
    return np.float32(total / (float(N) * float(N)))


def _run(x, y, trace=False):
    nc = _build()
    res = run_bass_kernel_spmd(nc, _in_maps(x, y), list(range(NCORES)),
                               trace=trace)
    return _combine(res.results, x, y), res


def kernel(x, y):
    x = np.asarray(x, dtype=np.float32)
    y = np.asarray(y, dtype=np.float32)
    loss, _ = _run(x, y, trace=False)
    return loss


# revision 50
# speedup vs baseline: 1.0506x; 1.0380x over previous
"""CrossCosineEmbeddingLoss kernel for 8 trn2 NeuronCores (v10).

loss = mean over all (i,j) of: 1 - cos(x_i, y_j) if i==j else relu(cos(x_i, y_j))

Identity: total = sum_ij relu(xhat_i . y_j) * rny_j
                + sum_i (1 - sim_ii - relu(sim_ii))
relu(c*s) = c*relu(s) for c>0, so y stays unnormalized through the matmul and
1/||y_j|| is applied per j after the i-sum. The device computes the O(n^2)
part: all-pairs dots + relu + per-j sums (8.4M MACs + 8.4M relu/adds per
core). The O(n*d) prep (norms, dtype cast, transposed layout) and the n-term
diagonal correction are host-side marshalling in numpy/fp64.

Sharding: rows of x across 8 cores (1024 each); y replicated. Both operands
are passed pre-transposed [d, rows] in bf16 so tiles DMA straight into the
matmul-ready layout (contraction dim d on partitions).

Per-core device program:
  - 2 DMAs for xhatT halves, 8 group DMAs for yT
  - main: 64 j-blocks: 2 bf16 matmuls (yT_t stationary, xhatT moving) ->
    [128,1024] fp32 PSUM (2 banks, bufs=4 = 4-deep pipeline) -> one
    relu+accum instruction into R[:, t], alternating ACT / DVE
    (measured ~1275ns per block per engine incl. accum-read ->
    ~41us window, the TRN2 ACT+DVE PSUM-read floor; DMA and GPSIMD
    cannot touch PSUM, and 16-bit PSUM matmul output is TRN3-only)
  - R [128, 64] DMAs out from the ACT queue (Sync's queue drains late).
"""

import numpy as np
import ml_dtypes

import concourse.bacc as bacc
import concourse.tile as tile
from concourse import mybir
from concourse.bass_utils import run_bass_kernel_spmd

N, D = 8192, 128
NCORES = 8
SH = N // NCORES          # 1024 rows of x per core
TX = SH // 128            # 8 x-tiles per core
TY = N // 128             # 64 y-tiles

f32 = mybir.dt.float32
bf16 = mybir.dt.bfloat16
AF = mybir.ActivationFunctionType
ALU = mybir.AluOpType

# main-loop reducer assignment: strict alternation ACT / DVE
ASSIGN = ["act" if t % 2 == 0 else "dve" for t in range(TY)]

_CACHE = {}


def _build():
    if "nc" in _CACHE:
        return _CACHE["nc"]
    nc = bacc.Bacc("TRN2", target_bir_lowering=False, debug=False,
                   num_devices=NCORES)
    xht_d = nc.dram_tensor("xht", [D, SH], bf16, kind="ExternalInput")
    ybt_d = nc.dram_tensor("ybt", [D, N], bf16, kind="ExternalInput")
    # TY per-block sums + 2 extra halves (blocks 0 and TY-1 are split
    # across both engines; host adds the halves)
    out_d = nc.dram_tensor("out", [128, TY + 2], f32, kind="ExternalOutput")

    with tile.TileContext(nc) as tc:
        with tc.tile_pool(name="singles", bufs=1) as singles:
            xhatT = singles.tile([128, TX, 128], bf16)
            yT = singles.tile([128, TY, 128], bf16)
            R = singles.tile([128, TY + 2], f32)
            garbage = singles.tile([128, 512], bf16)
            nc.vector.memset(garbage[:], 0)

            # ---- PE warm-up fillers: flip the HAM clock gate to 2.4 GHz
            # during the input-DMA dead zone so main matmuls start warm
            with tc.tile_pool(name="wpsum", bufs=1, space="PSUM") as wpsum:
                wp = wpsum.tile([128, 512], f32, tag="wp")
                for _ in range(12):
                    nc.tensor.matmul(wp[:], garbage[:, :128], garbage[:])

            # ---- input DMAs: xhatT halves first, then y groups
            flat = xhatT[:].rearrange("p a b -> p (a b)")
            nc.sync.dma_start(out=flat[:, :512], in_=xht_d[:, :512])
            nc.sync.dma_start(out=flat[:, 512:], in_=xht_d[:, 512:])
            # first two j-tiles arrive in tiny DMAs so blocks 0-1 start early
            nc.sync.dma_start(out=yT[:, 0, :], in_=ybt_d[:, 0:128])
            nc.sync.dma_start(out=yT[:, 1, :], in_=ybt_d[:, 128:256])
            nc.sync.dma_start(
                out=yT[:, 2:8, :],
                in_=ybt_d[:, 256:1024].rearrange("p (a b) -> p a b", b=128))
            for g in range(1, TY // 8):
                nc.sync.dma_start(
                    out=yT[:, 8 * g:8 * (g + 1), :],
                    in_=ybt_d[:, 1024 * g:1024 * (g + 1)]
                    .rearrange("p (a b) -> p a b", b=128))

            # ---- main: per j-block bf16 matmuls (fp32 PSUM, 2 banks,
            # bufs=4) + one relu+accum per block, ACT / DVE split
            with tc.tile_pool(name="mpsum", bufs=4, space="PSUM") as mpsum:
                for t in range(TY):
                    ps = mpsum.tile([128, 1024], f32, tag="mp")
                    lhsT = yT[:, t, :]
                    nc.tensor.matmul(ps[:, 0:512], lhsT, flat[:, 0:512])
                    nc.tensor.matmul(ps[:, 512:1024], lhsT,
                                     flat[:, 512:1024])
                    if t == 0 or t == TY - 1:
                        # edge blocks: both engines, half each, so the
                        # window starts and ends with no single-engine idle
                        ex = TY if t == 0 else TY + 1
                        nc.scalar.activation(
                            ps[:, :512], ps[:, :512], AF.Relu,
                            accum_out=R[:, t:t + 1])
                        nc.vector.tensor_scalar(
                            out=ps[:, 512:], in0=ps[:, 512:], scalar1=0.0,
                            scalar2=None, op0=ALU.max, op1=ALU.add,
                            accum_out=R[:, ex:ex + 1])
                    elif ASSIGN[t] == "act":
                        nc.scalar.activation(
                            ps[:], ps[:], AF.Relu,
                            accum_out=R[:, t:t + 1])
                    else:
                        nc.vector.tensor_scalar(
                            out=ps[:], in0=ps[:], scalar1=0.0,
                            scalar2=None, op0=ALU.max, op1=ALU.add,
                            accum_out=R[:, t:t + 1])

            # ---- output from the ACT queue (Sync's queue drains late;
            # gpsimd software-DGE measured slower)
            nc.scalar.dma_start(out=out_d[:], in_=R[:])

    nc.compile()
    _CACHE["nc"] = nc
    return nc


def _in_maps(x, y):
    bf = ml_dtypes.bfloat16
    rnx = 1.0 / np.maximum(np.sqrt((x.astype(np.float32) ** 2).sum(axis=1)),
                           1e-8)
    xhat = (x * rnx[:, None]).astype(bf)      # [N, D]
    xhT = np.ascontiguousarray(xhat.T)        # [D, N]
    ybt = np.ascontiguousarray(y.astype(bf).T)   # [D, N]
    maps = []
    for c in range(NCORES):
        sl = slice(SH * c, SH * (c + 1))
        maps.append({"xht": np.ascontiguousarray(xhT[:, sl]), "ybt": ybt})
    return maps


def _combine(results, x, y):
    x64 = x.astype(np.float64)
    y64 = y.astype(np.float64)
    ny = np.sqrt((y64 ** 2).sum(axis=1))
    rny = 1.0 / np.maximum(ny, 1e-8)          # [N]
    rny_pt = rny.reshape(TY, 128).T           # [128, TY], j = 128t + p
    total = 0.0
    for c in range(NCORES):
        Rx = results[c]["out"].astype(np.float64)     # [128, TY+2]
        R = Rx[:, :TY].copy()
        R[:, 0] += Rx[:, TY]          # second half of split block 0
        R[:, TY - 1] += Rx[:, TY + 1]  # second half of split block TY-1
        total += (R * rny_pt).sum()
    # diagonal correction in fp64 on host (n of n^2 terms)
    nx = np.sqrt((x64 ** 2).sum(axis=1))
    sim_d = (x64 * y64).sum(axis=1) / np.maximum(nx * ny, 1e-8)
    total += (1.0 - sim_d - np.maximum(sim_d, 0.0)).sum()
    return np.float32(total / (float(N) * float(N)))


def _run(x, y, trace=False):
    nc = _build()
    res = run_bass_kernel_spmd(nc, _in_maps(x, y), list(range(NCORES)),
                               trace=trace)
    return _combine(res.results, x, y), res


def kernel(x, y):
    x = np.asarray(x, dtype=np.float32)
    y = np.asarray(y, dtype=np.float32)
    loss, _ = _run(x, y, trace=False)
    return loss
